# revision 2
# baseline (speedup 1.0000x reference)
"""AttentionPool kernel for 8x Trainium2 NeuronCores (Bass/Tile).

Problem (per batch b of B=8):
    q = (x[:, :8] @ Wq.T).reshape(8, 24, 64) * 64**-0.5
    k = (x @ Wk.T).reshape(4096, 24, 64)
    v = (x @ Wv.T).reshape(4096, 24, 64)
    attn = softmax(mask(q @ k.T))          # [24, 8, 4096]
    out = (attn @ v).reshape(8, 1536) @ Wp.T + bp

Sharding: data-parallel over B - one batch per NeuronCore, no collectives.

Key algebraic restructuring (R=8 queries makes pooling tiny):
  logits[h*8+r, n] = q2[h*8+r, :] . x[n, :]   with q2 = (q*scale) @ Wk[head
      rows] folded on the host (38 MFLOP) -> the 19.3 GFLOP K-projection
      becomes a 2.4 GFLOP GEMM against x directly.
  pool p[hr, :] = sum_n e[hr, n] x[n, :] (unnormalized, 2.4 GFLOP), then
      x_cls[r, hb] = p[h*8+r, :] @ WvT[:, hb] / den[hr]  (38 MFLOP)
      out = x_cls @ WpT (38 MFLOP) -> the 19.3 GFLOP V-projection vanishes.
  Total device FLOPs drop ~8x; the kernel becomes DMA-bound on streaming x
  in two layouts (c-major for logits stationary, token-major for pool
  stationary) in fp16, ~25 MB at the modeled 360 GB/s.

Per-core dataflow (all matmuls fp16 (1 cyc/row at any moving size), psum
f32):
  phase 1 (per 512-token chunk): logits.T[tok, 192] per 128-token subtile
      (stationary = xT subtile, moving = q2T[ct]) -> exp (Act) -> * mask
      (DVE, broadcast over heads) -> eT[tok, 192] fp16 resident;
      den += ones.T @ eT (1-col stationary matmul);
      pool: per ct: psum[c128, 192] += x_sub.T-contract (stationary = x
      subtile [tok, c-cols], moving = eT) -> fp16 accumulator pT16.
  phase 2 (tail): recip(den) -> broadcast to 128 partitions via f32 matmul;
      v-apply: per ot: psum[128, 16] = sum_ct WvT-slab.T @ pT16[:, ct,
      head-pair cols]; normalize via recip in the psum->SBUF copy -> xclsT;
      out-proj: per ot: psum[128, 8] = sum_j WpT-slab.T @ xclsT[:, j] ->
      outT[1536, 8] -> host transposes + bias.
"""

import numpy as np

B, N, C = 8, 4096, 1536
H, HD, R = 24, 64, 8
HR = H * R           # 192 (h, r) pairs, index hr = h*R + r
SCALE = HD ** -0.5
P = 128
CT = C // P          # 12 contraction/output tiles
NCHUNK = 512
NSUB_CH = NCHUNK // P  # 4 subtiles per chunk
NT = N // NCHUNK     # 8 chunks
NSUB = N // P        # 32 token subtiles total

_RUNNER_CACHE = {}


def _build():
    import concourse.mybir as mybir
    import concourse.tile as tile
    from concourse import bacc

    F32 = mybir.dt.float32
    F16 = mybir.dt.float16
    MULT = mybir.AluOpType.mult
    EXP = mybir.ActivationFunctionType.Exp

    nc = bacc.Bacc(None, target_bir_lowering=False)
    xt = nc.dram_tensor("xt", [C, N], F16, kind="ExternalInput")      # x.T
    xn = nc.dram_tensor("xn", [N, C], F16, kind="ExternalInput")      # x
    q2t = nc.dram_tensor("q2t", [C, HR], F16, kind="ExternalInput")   # q2.T
    maskt = nc.dram_tensor("maskt", [N, R], F32, kind="ExternalInput")
    wvt = nc.dram_tensor("wvt", [C, C], F16, kind="ExternalInput")    # Wv.T
    wpt = nc.dram_tensor("wpt", [C, C], F16, kind="ExternalInput")    # Wp.T
    outt = nc.dram_tensor("outt", [C, R], F32, kind="ExternalOutput")  # out.T

    with tile.TileContext(nc) as tc:
        with (
            tc.tile_pool(name="pper", bufs=1) as pper,      # persistent
            tc.tile_pool(name="pxt", bufs=3) as pxt,        # xT chunks
            tc.tile_pool(name="pxn", bufs=3) as pxn,        # x chunks
            tc.tile_pool(name="pwv", bufs=1) as pwv,
            tc.tile_pool(name="pwp", bufs=1) as pwp,
            tc.tile_pool(name="pexp", bufs=2) as pexp,
            tc.tile_pool(name="psmall", bufs=1) as psmall,
            tc.tile_pool(name="ps_l", bufs=2, space="PSUM") as ps_l,
            tc.tile_pool(name="ps_p", bufs=4, space="PSUM") as ps_p,
            tc.tile_pool(name="ps_d", bufs=2, space="PSUM") as ps_d,
        ):
            # ---------- persistent tiles ----------
            q2t_sb = pper.tile([P, CT, HR], F16, tag="q2t")
            maskt_sb = pper.tile([P, NSUB, R], F32, tag="maskt")
            eT = pper.tile([P, NSUB, HR], F16, tag="eT")        # masked exp
            pT16 = pper.tile([P, CT, HR], F16, tag="pT")        # pool acc
            den_acc = pper.tile([1, HR], F32, tag="den")
            ones16 = pper.tile([P, 1], F16, tag="ones16")
            ones_row = pper.tile([1, P], F32, tag="onesrow")
            recip_bc = pper.tile([P, HR], F32, tag="recip")
            xcls16 = pper.tile([P, CT, R], F16, tag="xcls")
            outsb = pper.tile([P, CT, R], F32, tag="outsb")

            # ---------- startup DMAs (emission order = queue order) ----------
            # fine-grained first pieces so the first logits chain starts ~2us
            # in instead of waiting for whole tiles.
            def _q2t_slab(ct):
                nc.sync.dma_start(
                    q2t_sb[:, ct:ct + 1],
                    q2t[ct * P:(ct + 1) * P, :].rearrange(
                        "(ct p) hr -> p ct hr", p=P))

            xt_ch0 = pxt.tile([P, CT, NCHUNK], F16, tag="xt")

            def _xt_half(xt_ch, nt, half):
                lo = nt * NCHUNK + half * (NCHUNK // 2)
                nc.sync.dma_start(
                    xt_ch[:, :, half * (NCHUNK // 2):(half + 1) * (NCHUNK // 2)],
                    xt[:, lo:lo + NCHUNK // 2].rearrange(
                        "(ct p) n -> p ct n", p=P))

            xn_ch0 = pxn.tile([P, NSUB_CH, C], F16, tag="xn")

            def _xn_half(xn_ch, nt, half):
                lo = nt * NCHUNK + half * (NCHUNK // 2)
                nc.sync.dma_start(
                    xn_ch[:, half * 2:(half + 1) * 2],
                    xn[lo:lo + NCHUNK // 2, :].rearrange(
                        "(s p) c -> p s c", p=P))

            _q2t_slab(0)
            _xt_half(xt_ch0, 0, 0)
            for ct in range(1, CT):
                _q2t_slab(ct)
            nc.sync.dma_start(
                maskt_sb, maskt.rearrange("(s p) r -> p s r", p=P))
            _xn_half(xn_ch0, 0, 0)
            _xt_half(xt_ch0, 0, 1)
            _xn_half(xn_ch0, 0, 1)

            # ones vectors (fp16 via copy from f32 memset)
            ones_f = psmall.tile([P, 1], F32, tag="onesf")
            nc.vector.memset(ones_f, 1.0)
            nc.vector.tensor_copy(ones16, ones_f)
            nc.vector.memset(ones_row, 1.0)

            # W slabs interleave into the chunk stream below; emitted in
            # column quarters (768B runs) at chosen loop points.
            wvt_sb = pwv.tile([P, CT, C], F16, tag="wv")
            wpt_sb = pwp.tile([P, CT, C], F16, tag="wp")

            def _w_slab(dst_sb, src, quarter):
                w4 = C // 4
                nc.sync.dma_start(
                    dst_sb[:, :, quarter * w4:(quarter + 1) * w4],
                    src[:, quarter * w4:(quarter + 1) * w4].rearrange(
                        "(ct p) o -> p ct o", p=P))

            # ---------- phase 1: logits + exp*mask + den + pool ----------
            for nt in range(NT):
                if nt == 0:
                    xt_ch, xn_ch = xt_ch0, xn_ch0
                else:
                    xt_ch = pxt.tile([P, CT, NCHUNK], F16, tag="xt")
                    _xt_half(xt_ch, nt, 0)
                    _xt_half(xt_ch, nt, 1)
                    xn_ch = pxn.tile([P, NSUB_CH, C], F16, tag="xn")
                    _xn_half(xn_ch, nt, 0)
                    _xn_half(xn_ch, nt, 1)
                # spread the 8 W-slab loads across chunks 1..4 (two per
                # chunk) so they hide inside the stream but are long done
                # before the tail needs them.
                if 1 <= nt <= 2:
                    _w_slab(wvt_sb, wvt, 2 * (nt - 1))
                    _w_slab(wvt_sb, wvt, 2 * (nt - 1) + 1)
                elif 3 <= nt <= 4:
                    _w_slab(wpt_sb, wpt, 2 * (nt - 3))
                    _w_slab(wpt_sb, wpt, 2 * (nt - 3) + 1)

                # logits per subtile: lT[tok128, 192]
                den_ch = ps_d.tile([P, 512], F32, tag="pd")
                for s in range(NSUB_CH):
                    si = nt * NSUB_CH + s
                    ps = ps_l.tile([P, 512], F32, tag="pl")
                    lT = ps[:, 0:HR]
                    for ct in range(CT):
                        nc.tensor.matmul(
                            lT,
                            xt_ch[:, ct, s * P:(s + 1) * P],
                            q2t_sb[:, ct],
                            start=(ct == 0), stop=(ct == CT - 1))
                    exp_f = pexp.tile([P, HR], F32, tag="expf")
                    nc.scalar.activation(exp_f, lT, EXP)
                    nc.vector.tensor_tensor(
                        eT[:, si].rearrange("p (h r) -> p h r", h=H),
                        exp_f.rearrange("p (h r) -> p h r", h=H),
                        maskt_sb[:, si, None, :].to_broadcast((P, H, R)),
                        MULT)
                    nc.tensor.matmul(
                        den_ch[0:1, 0:HR], ones16, eT[:, si],
                        start=(s == 0), stop=(s == NSUB_CH - 1))
                if nt == 0:
                    nc.vector.tensor_copy(den_acc, den_ch[0:1, 0:HR])
                else:
                    nc.vector.tensor_add(
                        den_acc, den_acc, den_ch[0:1, 0:HR])

                # pool: pT[c128, 192] += x_sub.T @ eT_sub per c-tile
                for ct in range(CT):
                    ps = ps_p.tile([P, 512], F32, tag="pp")
                    pch = ps[:, 0:HR]
                    for s in range(NSUB_CH):
                        si = nt * NSUB_CH + s
                        nc.tensor.matmul(
                            pch,
                            xn_ch[:, s, ct * P:(ct + 1) * P],
                            eT[:, si],
                            start=(s == 0), stop=(s == NSUB_CH - 1))
                    if nt == 0:
                        nc.vector.tensor_copy(pT16[:, ct], pch)
                    else:
                        nc.vector.tensor_add(pT16[:, ct], pT16[:, ct], pch)

            # ---------- phase 2: normalize + V-apply + out-projection ----
            recip1 = psmall.tile([1, HR], F32, tag="recip1")
            nc.vector.reciprocal(recip1, den_acc)
            ps_bc = ps_d.tile([P, 512], F32, tag="pd")
            nc.tensor.matmul(
                ps_bc[:, 0:HR], ones_row, recip1, start=True, stop=True)
            nc.vector.tensor_copy(recip_bc, ps_bc[:, 0:HR])

            # v-apply: x_clsT[128, ot, r]; head 2*ot lives in rows 0:64
            # (psum cols 0:8), head 2*ot+1 in rows 64:128 (psum cols 8:16).
            # Normalization by 1/den folds into the psum->SBUF copy.
            for ot in range(CT):
                ps = ps_p.tile([P, 512], F32, tag="pp")
                pv = ps[:, 0:16]
                for ct in range(CT):
                    nc.tensor.matmul(
                        pv,
                        wvt_sb[:, ct, ot * P:(ot + 1) * P],
                        pT16[:, ct, 16 * ot:16 * ot + 16],
                        start=(ct == 0), stop=(ct == CT - 1))
                nc.vector.tensor_tensor(
                    xcls16[0:HD, ot], pv[0:HD, 0:R],
                    recip_bc[0:HD, 16 * ot:16 * ot + R], MULT)
                nc.vector.tensor_tensor(
                    xcls16[HD:P, ot], pv[HD:P, R:16],
                    recip_bc[HD:P, 16 * ot + R:16 * ot + 16], MULT)

            # out-projection: outT[128, ot2, r] = sum_j WpT-slab.T @ xclsT
            for ot2 in range(CT):
                ps = ps_l.tile([P, 512], F32, tag="pl")
                po = ps[:, 0:R]
                for j in range(CT):
                    nc.tensor.matmul(
                        po,
                        wpt_sb[:, j, ot2 * P:(ot2 + 1) * P],
                        xcls16[:, j],
                        start=(j == 0), stop=(j == CT - 1))
                nc.vector.tensor_copy(outsb[:, ot2], po)
            nc.sync.dma_start(
                outt.rearrange("(j p) r -> p j r", p=P), outsb)

    nc.compile()
    return nc


def _prep_inputs(x, mask, Wq, Wk, Wv, Wp, bp):
    """Host-side sharding + layout prep. Returns per-core in_maps.

    The 8-token q projection and its fold through Wk (q2 = q*scale @
    Wk[head rows]) happen here: 76 MFLOP of the 312 GFLOP problem, and
    doing it on-device would force 9.4 MB of Wq/Wk DMA for 0.02% of the
    FLOPs."""
    x = np.asarray(x, dtype=np.float32)
    Wq = np.asarray(Wq, np.float32)
    Wk = np.asarray(Wk, np.float32)
    wvt = np.ascontiguousarray(np.asarray(Wv, np.float32).T.astype(np.float16))
    wpt = np.ascontiguousarray(np.asarray(Wp, np.float32).T.astype(np.float16))

    mask = np.asarray(mask)
    mask_full = np.empty((B, R, N), np.float32)
    mask_full[:, :, :R] = np.eye(R, dtype=np.float32)[None]
    mask_full[:, :, R:] = mask.astype(np.float32)

    # q2[b, hr, c] = sum_d q[b, r, h, d]*SCALE * Wk[h*HD+d, c]
    q = np.einsum('brc,dc->brd', x[:, :R], Wq) * SCALE        # [B, R, C]
    q2 = np.einsum('brhd,hdc->bhrc',
                   q.reshape(B, R, H, HD), Wk.reshape(H, HD, C))
    q2 = q2.reshape(B, HR, C)

    in_maps = []
    for b in range(B):
        xt_b = np.ascontiguousarray(x[b].T.astype(np.float16))
        xn_b = np.ascontiguousarray(x[b].astype(np.float16))
        q2t_b = np.ascontiguousarray(q2[b].T.astype(np.float16))
        maskt_b = np.ascontiguousarray(mask_full[b].T)
        in_maps.append({
            "xt": xt_b, "xn": xn_b, "q2t": q2t_b, "maskt": maskt_b,
            "wvt": wvt, "wpt": wpt,
        })
    return in_maps


def _get_nc():
    if "nc" not in _RUNNER_CACHE:
        _RUNNER_CACHE["nc"] = _build()
    return _RUNNER_CACHE["nc"]


def kernel(x, mask, Wq, Wk, Wv, Wp, bp, repeats=8, **_unused):
    from concourse import bass_utils

    in_maps = _prep_inputs(x, mask, Wq, Wk, Wv, Wp, bp)
    nc = _get_nc()
    res = bass_utils.run_bass_kernel_spmd(nc, in_maps, core_ids=list(range(B)))
    out = np.stack(
        [res.results[b]["outt"].T for b in range(B)], axis=0)
    out = out + np.asarray(bp, np.float32).reshape(1, 1, C)
    return out.astype(np.float32)


if __name__ == "__main__":
    rng = np.random.default_rng(0)
    x = rng.standard_normal((B, N, C)).astype(np.float32)
    mask = rng.integers(0, 2, (B, R, N - R)) > 0
    s = 0.02
    Wq = (rng.standard_normal((C, C)) * s).astype(np.float32)
    Wk = (rng.standard_normal((C, C)) * s).astype(np.float32)
    Wv = (rng.standard_normal((C, C)) * s).astype(np.float32)
    Wp = (rng.standard_normal((C, C)) * s).astype(np.float32)
    bp = np.zeros(C, np.float32)
    out = kernel(x, mask, Wq, Wk, Wv, Wp, bp, 8)
    print("out", out.shape, out.dtype, np.abs(out).mean())


# revision 3
# speedup vs baseline: 1.0255x; 1.0255x over previous
"""AttentionPool kernel for 8x Trainium2 NeuronCores (Bass/Tile).

Problem (per batch b of B=8):
    q = (x[:, :8] @ Wq.T).reshape(8, 24, 64) * 64**-0.5
    k = (x @ Wk.T).reshape(4096, 24, 64)
    v = (x @ Wv.T).reshape(4096, 24, 64)
    attn = softmax(mask(q @ k.T))          # [24, 8, 4096]
    out = (attn @ v).reshape(8, 1536) @ Wp.T + bp

Sharding: data-parallel over B - one batch per NeuronCore, no collectives.

Key algebraic restructuring (R=8 queries makes pooling tiny):
  logits[h*8+r, n] = q2[h*8+r, :] . x[n, :]   with q2 = (q*scale) @ Wk[head
      rows] folded on the host (76 MFLOP) -> the 19.3 GFLOP K-projection
      becomes a 2.4 GFLOP GEMM against x directly.
  pool p[hr, :] = sum_n e[hr, n] x[n, :] (unnormalized, 2.4 GFLOP), then
      x_cls[r, hb] = p[h*8+r, :] @ WvT[:, hb] / den[hr]  (38 MFLOP)
      out = x_cls @ WpT (38 MFLOP) -> the 19.3 GFLOP V-projection vanishes.
  Total device FLOPs drop ~8x; the kernel becomes DMA-bound on streaming x
  in two layouts (c-major for logits stationary, token-major for pool
  stationary) in fp16, ~25 MB at the modeled 360 GB/s.

Per-core dataflow (all matmuls fp16 = 1 cyc/row at any moving size, psum
f32). DMA queue order == emission order; the stream is packed so the DMA
engines run gapless while PE/DVE/Act trail it by under a chunk:
  per 512-token chunk: logits.T[tok, 192] per 128-token subtile (stationary
      = xT subtile, moving = q2T[ct]) -> exp (Act) -> * mask (DVE broadcast
      over heads) -> eT[tok, 192] fp16; pool per c-tile: psum[c128, 192] +=
      x_sub.T-contract (stationary = x subtile, moving = eT) -> fp16
      accumulators pT_a (chunks 0-3) / pT_b (chunks 4-7); den via 1-col
      ones stationary after each chunk's pool.
  v-apply runs in two passes so only the second sits in the tail:
      pass A (after chunk 3, overlapped with chunk-4 streaming) and pass B
      (tail) each do 12x12 matmuls of WvT-slab.T @ pT[:, ct, head-pair
      cols] -> x_cls accumulator; normalize by 1/den (broadcast via f32
      matmul) folds into the final copy -> xclsT fp16.
  out-proj: per cout tile: psum[128, 8] = sum_j WpT-slab.T @ xclsT[:, j]
      -> outT[1536, 8] f32 -> host transposes + bias.
"""

import numpy as np

B, N, C = 8, 4096, 1536
H, HD, R = 24, 64, 8
HR = H * R           # 192 (h, r) pairs, index hr = h*R + r
HRP = 256            # q2t free-dim padded so DMA runs are 512B
SCALE = HD ** -0.5
P = 128
CT = C // P          # 12 contraction/output tiles
NCHUNK = 512
NSUB_CH = NCHUNK // P  # 4 subtiles per chunk
NT = N // NCHUNK     # 8 chunks
NSUB = N // P        # 32 token subtiles total

_RUNNER_CACHE = {}


def _build():
    import concourse.mybir as mybir
    import concourse.tile as tile
    from concourse import bacc

    F32 = mybir.dt.float32
    F16 = mybir.dt.float16
    MULT = mybir.AluOpType.mult
    EXP = mybir.ActivationFunctionType.Exp

    nc = bacc.Bacc(None, target_bir_lowering=False)
    xt = nc.dram_tensor("xt", [C, N], F16, kind="ExternalInput")      # x.T
    xn = nc.dram_tensor("xn", [N, C], F16, kind="ExternalInput")      # x
    q2t = nc.dram_tensor("q2t", [C, HRP], F16, kind="ExternalInput")  # q2.T
    maskt = nc.dram_tensor("maskt", [N, R], F32, kind="ExternalInput")
    wvt = nc.dram_tensor("wvt", [C, C], F16, kind="ExternalInput")    # Wv.T
    wpt = nc.dram_tensor("wpt", [C, C], F16, kind="ExternalInput")    # Wp.T
    outt = nc.dram_tensor("outt", [C, R], F32, kind="ExternalOutput")  # out.T

    with tile.TileContext(nc) as tc:
        with (
            tc.tile_pool(name="pper", bufs=1) as pper,      # persistent
            tc.tile_pool(name="pxt", bufs=3) as pxt,        # xT chunks
            tc.tile_pool(name="pxn", bufs=3) as pxn,        # x chunks
            tc.tile_pool(name="pwv", bufs=1) as pwv,
            tc.tile_pool(name="pwp", bufs=1) as pwp,
            tc.tile_pool(name="pexp", bufs=2) as pexp,
            tc.tile_pool(name="psmall", bufs=1) as psmall,
            tc.tile_pool(name="ps_l", bufs=2, space="PSUM") as ps_l,
            tc.tile_pool(name="ps_p", bufs=4, space="PSUM") as ps_p,
            tc.tile_pool(name="ps_d", bufs=2, space="PSUM") as ps_d,
        ):
            # ---------- persistent tiles ----------
            q2t_sb = pper.tile([P, CT, HRP], F16, tag="q2t")
            maskt_sb = pper.tile([P, NSUB, R], F32, tag="maskt")
            eT = pper.tile([P, NSUB, HR], F16, tag="eT")        # masked exp
            pT_a = pper.tile([P, CT, HR], F16, tag="pTa")       # chunks 0-3
            pT_b = pper.tile([P, CT, HR], F16, tag="pTb")       # chunks 4-7
            den_acc = pper.tile([1, HR], F32, tag="den")
            ones16 = pper.tile([P, 1], F16, tag="ones16")
            ones_row = pper.tile([1, P], F32, tag="onesrow")
            recip_bc = pper.tile([P, HR], F32, tag="recip")
            xcls_acc = pper.tile([P, CT, 16], F32, tag="xacc")  # unnormalized
            xcls16 = pper.tile([P, CT, R], F16, tag="xcls")
            outsb = pper.tile([P, CT, R], F32, tag="outsb")

            # ---------- DMA emission helpers (order == queue order) -------
            xt_ch0 = pxt.tile([P, CT, NCHUNK], F16, tag="xt")

            def _xt_half(xt_ch, nt, half):
                lo = nt * NCHUNK + half * (NCHUNK // 2)
                nc.sync.dma_start(
                    xt_ch[:, :, half * (NCHUNK // 2):(half + 1) * (NCHUNK // 2)],
                    xt[:, lo:lo + NCHUNK // 2].rearrange(
                        "(ct p) n -> p ct n", p=P))

            xn_ch0 = pxn.tile([P, NSUB_CH, C], F16, tag="xn")

            def _xn_half(xn_ch, nt, half):
                lo = nt * NCHUNK + half * (NCHUNK // 2)
                nc.sync.dma_start(
                    xn_ch[:, half * 2:(half + 1) * 2],
                    xn[lo:lo + NCHUNK // 2, :].rearrange(
                        "(s p) c -> p s c", p=P))

            wvt_sb = pwv.tile([P, CT, C], F16, tag="wv")
            wpt_sb = pwp.tile([P, CT, C], F16, tag="wp")

            def _w_slab(dst_sb, src, quarter):
                w4 = C // 4
                nc.sync.dma_start(
                    dst_sb[:, :, quarter * w4:(quarter + 1) * w4],
                    src[:, quarter * w4:(quarter + 1) * w4].rearrange(
                        "(ct p) o -> p ct o", p=P))

            # startup: first xt half, q2t, rest of chunk 0, then Wv so
            # v-apply pass A never stalls the in-order PE stream.
            _xt_half(xt_ch0, 0, 0)
            nc.sync.dma_start(
                q2t_sb, q2t.rearrange("(ct p) hr -> p ct hr", p=P))
            _xt_half(xt_ch0, 0, 1)
            nc.sync.dma_start(
                maskt_sb, maskt.rearrange("(s p) r -> p s r", p=P))
            _xn_half(xn_ch0, 0, 0)
            _xn_half(xn_ch0, 0, 1)
            for quarter in range(4):
                _w_slab(wvt_sb, wvt, quarter)

            # ones vectors (fp16 via copy from f32 memset)
            ones_f = psmall.tile([P, 1], F32, tag="onesf")
            nc.vector.memset(ones_f, 1.0)
            nc.vector.tensor_copy(ones16, ones_f)
            nc.vector.memset(ones_row, 1.0)

            # ---------- per-chunk pipeline ----------
            def emit_logits(nt, xt_ch):
                for s in range(NSUB_CH):
                    si = nt * NSUB_CH + s
                    ps = ps_l.tile([P, 512], F32, tag="pl")
                    lT = ps[:, 0:HR]
                    for ct in range(CT):
                        nc.tensor.matmul(
                            lT,
                            xt_ch[:, ct, s * P:(s + 1) * P],
                            q2t_sb[:, ct, 0:HR],
                            start=(ct == 0), stop=(ct == CT - 1))
                    exp_f = pexp.tile([P, HR], F32, tag="expf")
                    nc.scalar.activation(exp_f, lT, EXP)
                    nc.vector.tensor_tensor(
                        eT[:, si].rearrange("p (h r) -> p h r", h=H),
                        exp_f.rearrange("p (h r) -> p h r", h=H),
                        maskt_sb[:, si, None, :].to_broadcast((P, H, R)),
                        MULT)

            def emit_pool(nt, xn_ch, pT, s_lo, s_hi, first):
                # pool psum groups over subtiles [s_lo, s_hi) of this chunk
                for ct in range(CT):
                    ps = ps_p.tile([P, 512], F32, tag="pp")
                    pch = ps[:, 0:HR]
                    for s in range(s_lo, s_hi):
                        si = nt * NSUB_CH + s
                        nc.tensor.matmul(
                            pch,
                            xn_ch[:, s, ct * P:(ct + 1) * P],
                            eT[:, si],
                            start=(s == s_lo), stop=(s == s_hi - 1))
                    if first:
                        nc.vector.tensor_copy(pT[:, ct], pch)
                    else:
                        nc.vector.tensor_add(pT[:, ct], pT[:, ct], pch)

            def emit_den(nt):
                den_ch = ps_d.tile([P, 512], F32, tag="pd")
                for s in range(NSUB_CH):
                    si = nt * NSUB_CH + s
                    nc.tensor.matmul(
                        den_ch[0:1, 0:HR], ones16, eT[:, si],
                        start=(s == 0), stop=(s == NSUB_CH - 1))
                if nt == 0:
                    nc.vector.tensor_copy(den_acc, den_ch[0:1, 0:HR])
                else:
                    nc.vector.tensor_add(
                        den_acc, den_acc, den_ch[0:1, 0:HR])

            def emit_vapply(pT, first):
                # x_cls accumulation: per cout tile ot, heads (2ot, 2ot+1)
                # live in pT columns 16ot..16ot+16; junk-free block-diagonal
                # moving keeps this at 144 matmuls x 16 rows.
                for ot in range(CT):
                    ps = ps_l.tile([P, 512], F32, tag="pl")
                    pv = ps[:, 0:16]
                    for ct in range(CT):
                        nc.tensor.matmul(
                            pv,
                            wvt_sb[:, ct, ot * P:(ot + 1) * P],
                            pT[:, ct, 16 * ot:16 * ot + 16],
                            start=(ct == 0), stop=(ct == CT - 1))
                    if first:
                        nc.vector.tensor_copy(xcls_acc[:, ot], pv)
                    else:
                        nc.vector.tensor_add(
                            xcls_acc[:, ot], xcls_acc[:, ot], pv)

            for nt in range(NT):
                if nt == 0:
                    xt_ch, xn_ch = xt_ch0, xn_ch0
                else:
                    xt_ch = pxt.tile([P, CT, NCHUNK], F16, tag="xt")
                    _xt_half(xt_ch, nt, 0)
                    _xt_half(xt_ch, nt, 1)
                    xn_ch = pxn.tile([P, NSUB_CH, C], F16, tag="xn")
                    _xn_half(xn_ch, nt, 0)
                    _xn_half(xn_ch, nt, 1)
                if 1 <= nt <= 4:
                    _w_slab(wpt_sb, wpt, nt - 1)

                emit_logits(nt, xt_ch)
                pT = pT_a if nt < 4 else pT_b
                if nt == NT - 1:
                    # split the final chunk's pool so the tail only waits on
                    # the last 256 tokens
                    emit_pool(nt, xn_ch, pT, 0, 2, first=(nt == 4))
                    emit_pool(nt, xn_ch, pT, 2, 4, first=False)
                else:
                    emit_pool(nt, xn_ch, pT, 0, NSUB_CH, first=(nt % 4 == 0))
                emit_den(nt)
                if nt == 3:
                    # pass A overlaps chunk-4 streaming
                    emit_vapply(pT_a, first=True)

            # ---------- tail ----------
            recip1 = psmall.tile([1, HR], F32, tag="recip1")
            nc.vector.reciprocal(recip1, den_acc)
            ps_bc = ps_d.tile([P, 512], F32, tag="pd")
            nc.tensor.matmul(
                ps_bc[:, 0:HR], ones_row, recip1, start=True, stop=True)
            nc.vector.tensor_copy(recip_bc, ps_bc[:, 0:HR])

            emit_vapply(pT_b, first=False)

            # normalize into fp16: head 2ot in rows 0:64 (cols 0:8), head
            # 2ot+1 in rows 64:128 (cols 8:16)
            for ot in range(CT):
                nc.vector.tensor_tensor(
                    xcls16[0:HD, ot], xcls_acc[0:HD, ot, 0:R],
                    recip_bc[0:HD, 16 * ot:16 * ot + R], MULT)
                nc.vector.tensor_tensor(
                    xcls16[HD:P, ot], xcls_acc[HD:P, ot, R:16],
                    recip_bc[HD:P, 16 * ot + R:16 * ot + 16], MULT)

            # out-projection: outT[128, ot2, r] = sum_j WpT-slab.T @ xclsT
            for ot2 in range(CT):
                ps = ps_p.tile([P, 512], F32, tag="pp")
                po = ps[:, 0:R]
                for j in range(CT):
                    nc.tensor.matmul(
                        po,
                        wpt_sb[:, j, ot2 * P:(ot2 + 1) * P],
                        xcls16[:, j],
                        start=(j == 0), stop=(j == CT - 1))
                nc.vector.tensor_copy(outsb[:, ot2], po)
            nc.sync.dma_start(
                outt.rearrange("(j p) r -> p j r", p=P), outsb)

    nc.compile()
    return nc


def _prep_inputs(x, mask, Wq, Wk, Wv, Wp, bp):
    """Host-side sharding + layout prep. Returns per-core in_maps.

    The 8-token q projection and its fold through Wk (q2 = q*scale @
    Wk[head rows]) happen here: 76 MFLOP of the 312 GFLOP problem, and
    doing it on-device would force 9.4 MB of Wq/Wk DMA for 0.02% of the
    FLOPs."""
    x = np.asarray(x, dtype=np.float32)
    Wq = np.asarray(Wq, np.float32)
    Wk = np.asarray(Wk, np.float32)
    wvt = np.ascontiguousarray(np.asarray(Wv, np.float32).T.astype(np.float16))
    wpt = np.ascontiguousarray(np.asarray(Wp, np.float32).T.astype(np.float16))

    mask = np.asarray(mask)
    mask_full = np.empty((B, R, N), np.float32)
    mask_full[:, :, :R] = np.eye(R, dtype=np.float32)[None]
    mask_full[:, :, R:] = mask.astype(np.float32)

    # q2[b, hr, c] = sum_d q[b, r, h, d]*SCALE * Wk[h*HD+d, c]
    q = np.einsum('brc,dc->brd', x[:, :R], Wq) * SCALE        # [B, R, C]
    q2 = np.einsum('brhd,hdc->bhrc',
                   q.reshape(B, R, H, HD), Wk.reshape(H, HD, C))
    q2 = q2.reshape(B, HR, C)

    in_maps = []
    for b in range(B):
        xt_b = np.ascontiguousarray(x[b].T.astype(np.float16))
        xn_b = np.ascontiguousarray(x[b].astype(np.float16))
        q2t_b = np.zeros((C, HRP), np.float16)
        q2t_b[:, 0:HR] = q2[b].T.astype(np.float16)
        maskt_b = np.ascontiguousarray(mask_full[b].T)
        in_maps.append({
            "xt": xt_b, "xn": xn_b, "q2t": q2t_b, "maskt": maskt_b,
            "wvt": wvt, "wpt": wpt,
        })
    return in_maps


def _get_nc():
    if "nc" not in _RUNNER_CACHE:
        _RUNNER_CACHE["nc"] = _build()
    return _RUNNER_CACHE["nc"]


def kernel(x, mask, Wq, Wk, Wv, Wp, bp, repeats=8, **_unused):
    from concourse import bass_utils

    in_maps = _prep_inputs(x, mask, Wq, Wk, Wv, Wp, bp)
    nc = _get_nc()
    res = bass_utils.run_bass_kernel_spmd(nc, in_maps, core_ids=list(range(B)))
    out = np.stack(
        [res.results[b]["outt"].T for b in range(B)], axis=0)
    out = out + np.asarray(bp, np.float32).reshape(1, 1, C)
    return out.astype(np.float32)


if __name__ == "__main__":
    rng = np.random.default_rng(0)
    x = rng.standard_normal((B, N, C)).astype(np.float32)
    mask = rng.integers(0, 2, (B, R, N - R)) > 0
    s = 0.02
    Wq = (rng.standard_normal((C, C)) * s).astype(np.float32)
    Wk = (rng.standard_normal((C, C)) * s).astype(np.float32)
    Wv = (rng.standard_normal((C, C)) * s).astype(np.float32)
    Wp = (rng.standard_normal((C, C)) * s).astype(np.float32)
    bp = np.zeros(C, np.float32)
    out = kernel(x, mask, Wq, Wk, Wv, Wp, bp, 8)
    print("out", out.shape, out.dtype, np.abs(out).mean())


# revision 5
# speedup vs baseline: 1.0587x; 1.0324x over previous
"""AttentionPool kernel for 8x Trainium2 NeuronCores (Bass/Tile).

Problem (per batch b of B=8):
    q = (x[:, :8] @ Wq.T).reshape(8, 24, 64) * 64**-0.5
    k = (x @ Wk.T).reshape(4096, 24, 64)
    v = (x @ Wv.T).reshape(4096, 24, 64)
    attn = softmax(mask(q @ k.T))          # [24, 8, 4096]
    out = (attn @ v).reshape(8, 1536) @ Wp.T + bp

Sharding: data-parallel over B - one batch per NeuronCore, no collectives.

Key algebraic restructuring (R=8 queries makes pooling tiny):
  logits[h*8+r, n] = q2[h*8+r, :] . x[n, :]   with q2 = (q*scale) @ Wk[head
      rows] folded on the host (76 MFLOP) -> the 19.3 GFLOP K-projection
      becomes a 2.4 GFLOP GEMM against x directly.
  pool p[hr, :] = sum_n e[hr, n] x[n, :] (unnormalized, 2.4 GFLOP), then
      x_cls[r, hb] = p[h*8+r, :] @ WvT[:, hb] / den[hr]  (38 MFLOP)
      out = x_cls @ WpT (38 MFLOP) -> the 19.3 GFLOP V-projection vanishes.
  Total device FLOPs drop ~8x; the kernel becomes DMA-bound on streaming x
  in two layouts (c-major for logits stationary, token-major for pool
  stationary) in fp16, ~25 MB at the modeled 360 GB/s.

Schedule (DMA queue order == emission order; the stream is packed so the
DMA engines run gapless; WpT is loaded LAST so the pool/v-apply/normalize
tail hides under its transfer and out-proj co-streams with its arrival):
  per 512-token chunk: logits.T[tok, 192] per 128-token subtile (stationary
      = xT subtile, moving = q2T[ct]) -> exp (Act) -> * mask (DVE broadcast
      over heads) -> eT fp16; pool per c-tile: psum[c128, 192] accumulated
      over the chunk -> fp16 slabs pT (one per 2 chunks, copy+add drains);
      den via 1-col ones stationary after each chunk's pool.
  v-apply in 4 passes (after chunks 1/3/5/7): 12x12 matmuls of
      WvT-slab.T @ pT[:, ct, head-pair cols], all accumulating into ONE
      dedicated psum bank across passes (start only on the very first
      matmul; psum has_written bits make later regions/passes accumulate
      correctly) -> no SBUF accumulator traffic at all.
  tail: recip(den) broadcast via f32 matmul; normalize psum -> xclsT fp16
      (head 2t in rows 0:64/cols 0:8, head 2t+1 in rows 64:128/cols 8:16);
      out-proj per cout tile: psum[128, 8] = sum_j WpT-slab.T @ xclsT[:, j]
      -> outT[1536, 8] f32 -> host transposes + bias.
"""

import numpy as np

B, N, C = 8, 4096, 1536
H, HD, R = 24, 64, 8
HR = H * R           # 192 (h, r) pairs, index hr = h*R + r
HRP = 256            # q2t free-dim padded so DMA runs are 512B
SCALE = HD ** -0.5
P = 128
CT = C // P          # 12 contraction/output tiles
NCHUNK = 512
NSUB_CH = NCHUNK // P  # 4 subtiles per chunk
NT = N // NCHUNK     # 8 chunks
NSUB = N // P        # 32 token subtiles total

_RUNNER_CACHE = {}


def _build():
    import concourse.mybir as mybir
    import concourse.tile as tile
    from concourse import bacc

    F32 = mybir.dt.float32
    F16 = mybir.dt.float16
    MULT = mybir.AluOpType.mult
    EXP = mybir.ActivationFunctionType.Exp

    nc = bacc.Bacc(None, target_bir_lowering=False)
    xt = nc.dram_tensor("xt", [C, N], F16, kind="ExternalInput")      # x.T
    xn = nc.dram_tensor("xn", [N, C], F16, kind="ExternalInput")      # x
    q2t = nc.dram_tensor("q2t", [C, HRP], F16, kind="ExternalInput")  # q2.T
    maskt = nc.dram_tensor("maskt", [N, R], F32, kind="ExternalInput")
    wvt = nc.dram_tensor("wvt", [C, C], F16, kind="ExternalInput")    # Wv.T
    wpt = nc.dram_tensor("wpt", [C, C], F16, kind="ExternalInput")    # Wp.T
    outt = nc.dram_tensor("outt", [C, R], F32, kind="ExternalOutput")  # out.T

    with tile.TileContext(nc) as tc:
        with (
            tc.tile_pool(name="pper", bufs=1) as pper,      # persistent
            tc.tile_pool(name="pxt", bufs=3) as pxt,        # xT chunks
            tc.tile_pool(name="pxn", bufs=3) as pxn,        # x chunks
            tc.tile_pool(name="pwv", bufs=1) as pwv,
            tc.tile_pool(name="pwp", bufs=1) as pwp,
            tc.tile_pool(name="pexp", bufs=2) as pexp,
            tc.tile_pool(name="psmall", bufs=1) as psmall,
            tc.tile_pool(name="ps_l", bufs=2, space="PSUM") as ps_l,
            tc.tile_pool(name="ps_p", bufs=3, space="PSUM") as ps_p,
            tc.tile_pool(name="ps_d", bufs=1, space="PSUM") as ps_d,
            tc.tile_pool(name="ps_x", bufs=1, space="PSUM") as ps_x,
        ):
            # ---------- persistent tiles ----------
            q2t_sb = pper.tile([P, CT, HRP], F16, tag="q2t")
            maskt_sb = pper.tile([P, NSUB, R], F32, tag="maskt")
            eT = pper.tile([P, NSUB, HR], F16, tag="eT")        # masked exp
            pT = [pper.tile([P, CT, HR], F16, tag=f"pT{i}", name=f"pT{i}")
                  for i in range(4)]
            den_acc = pper.tile([1, HR], F32, tag="den")
            ones16 = pper.tile([P, 1], F16, tag="ones16")
            ones_row = pper.tile([1, P], F32, tag="onesrow")
            recip_bc = pper.tile([P, HR], F32, tag="recip")
            xcls16 = pper.tile([P, CT, R], F16, tag="xcls")
            outsb = pper.tile([P, CT, R], F32, tag="outsb")
            # single psum bank accumulating x_cls across all 4 v-apply passes
            xc_ps = ps_x.tile([P, 512], F32, tag="px")

            # ---------- DMA emission helpers (order == queue order) -------
            xt_ch0 = pxt.tile([P, CT, NCHUNK], F16, tag="xt")

            def _xt_half(xt_ch, nt, half):
                lo = nt * NCHUNK + half * (NCHUNK // 2)
                nc.sync.dma_start(
                    xt_ch[:, :, half * (NCHUNK // 2):(half + 1) * (NCHUNK // 2)],
                    xt[:, lo:lo + NCHUNK // 2].rearrange(
                        "(ct p) n -> p ct n", p=P))

            xn_ch0 = pxn.tile([P, NSUB_CH, C], F16, tag="xn")

            def _xn_half(xn_ch, nt, half):
                lo = nt * NCHUNK + half * (NCHUNK // 2)
                nc.sync.dma_start(
                    xn_ch[:, half * 2:(half + 1) * 2],
                    xn[lo:lo + NCHUNK // 2, :].rearrange(
                        "(s p) c -> p s c", p=P))

            wvt_sb = pwv.tile([P, CT, C], F16, tag="wv")
            wpt_sb = pwp.tile([P, CT, C], F16, tag="wp")

            def _w_slab(dst_sb, src, quarter):
                w4 = C // 4
                nc.sync.dma_start(
                    dst_sb[:, :, quarter * w4:(quarter + 1) * w4],
                    src[:, quarter * w4:(quarter + 1) * w4].rearrange(
                        "(ct p) o -> p ct o", p=P))

            # startup: chunk 0 + q2t + mask, then Wv (needed by the first
            # v-apply pass after chunk 1). WpT is NOT here - it loads at the
            # very end of the stream.
            _xt_half(xt_ch0, 0, 0)
            nc.sync.dma_start(
                q2t_sb, q2t.rearrange("(ct p) hr -> p ct hr", p=P))
            _xt_half(xt_ch0, 0, 1)
            nc.sync.dma_start(
                maskt_sb, maskt.rearrange("(s p) r -> p s r", p=P))
            _xn_half(xn_ch0, 0, 0)
            _xn_half(xn_ch0, 0, 1)
            for quarter in range(4):
                _w_slab(wvt_sb, wvt, quarter)

            # ones vectors (fp16 via copy from f32 memset)
            ones_f = psmall.tile([P, 1], F32, tag="onesf")
            nc.vector.memset(ones_f, 1.0)
            nc.vector.tensor_copy(ones16, ones_f)
            nc.vector.memset(ones_row, 1.0)

            # ---------- per-chunk pipeline ----------
            def emit_logits(nt, xt_ch):
                for s in range(NSUB_CH):
                    si = nt * NSUB_CH + s
                    ps = ps_l.tile([P, 512], F32, tag="pl")
                    lT = ps[:, 0:HR]
                    for ct in range(CT):
                        nc.tensor.matmul(
                            lT,
                            xt_ch[:, ct, s * P:(s + 1) * P],
                            q2t_sb[:, ct, 0:HR],
                            start=(ct == 0), stop=(ct == CT - 1))
                    exp_f = pexp.tile([P, HR], F32, tag="expf")
                    nc.scalar.activation(exp_f, lT, EXP)
                    nc.vector.tensor_tensor(
                        eT[:, si].rearrange("p (h r) -> p h r", h=H),
                        exp_f.rearrange("p (h r) -> p h r", h=H),
                        maskt_sb[:, si, None, :].to_broadcast((P, H, R)),
                        MULT)

            def emit_pool(nt, xn_ch):
                slab = pT[nt // 2]
                for ct in range(CT):
                    ps = ps_p.tile([P, 512], F32, tag="pp")
                    pch = ps[:, 0:HR]
                    for s in range(NSUB_CH):
                        si = nt * NSUB_CH + s
                        nc.tensor.matmul(
                            pch,
                            xn_ch[:, s, ct * P:(ct + 1) * P],
                            eT[:, si],
                            start=(s == 0), stop=(s == NSUB_CH - 1))
                    if nt % 2 == 0:
                        nc.vector.tensor_copy(slab[:, ct], pch)
                    else:
                        nc.vector.tensor_add(slab[:, ct], slab[:, ct], pch)

            def emit_den(nt):
                den_ch = ps_d.tile([P, 512], F32, tag="pd")
                for s in range(NSUB_CH):
                    si = nt * NSUB_CH + s
                    nc.tensor.matmul(
                        den_ch[0:1, 0:HR], ones16, eT[:, si],
                        start=(s == 0), stop=(s == NSUB_CH - 1))
                if nt == 0:
                    nc.vector.tensor_copy(den_acc, den_ch[0:1, 0:HR])
                else:
                    nc.vector.tensor_add(
                        den_acc, den_acc, den_ch[0:1, 0:HR])

            def emit_vapply(pass_i):
                # x_cls accumulation into the dedicated psum bank. start
                # only on the very first matmul (clears the bank once);
                # every other region/pass relies on psum has_written bits:
                # first write to a fresh region replaces, later ones
                # accumulate. stop only on the final pass.
                slab = pT[pass_i]
                for ot in range(CT):
                    pv = xc_ps[:, 16 * ot:16 * ot + 16]
                    for ct in range(CT):
                        nc.tensor.matmul(
                            pv,
                            wvt_sb[:, ct, ot * P:(ot + 1) * P],
                            slab[:, ct, 16 * ot:16 * ot + 16],
                            start=(pass_i == 0 and ot == 0 and ct == 0),
                            stop=(pass_i == 3 and ct == CT - 1),
                            skip_group_check=True)

            for nt in range(NT):
                if nt == 0:
                    xt_ch, xn_ch = xt_ch0, xn_ch0
                else:
                    xt_ch = pxt.tile([P, CT, NCHUNK], F16, tag="xt")
                    _xt_half(xt_ch, nt, 0)
                    _xt_half(xt_ch, nt, 1)
                    xn_ch = pxn.tile([P, NSUB_CH, C], F16, tag="xn")
                    _xn_half(xn_ch, nt, 0)
                    _xn_half(xn_ch, nt, 1)
                emit_logits(nt, xt_ch)
                emit_pool(nt, xn_ch)
                emit_den(nt)
                if nt % 2 == 1:
                    emit_vapply(nt // 2)

            # WpT loads at the very end of the DMA stream: the whole
            # pool/v-apply/normalize tail hides under its transfer and the
            # out-projection co-streams with its column quarters.
            for quarter in range(4):
                _w_slab(wpt_sb, wpt, quarter)

            # ---------- tail ----------
            recip1 = psmall.tile([1, HR], F32, tag="recip1")
            nc.vector.reciprocal(recip1, den_acc)
            ps_bc = ps_d.tile([P, 512], F32, tag="pd")
            nc.tensor.matmul(
                ps_bc[:, 0:HR], ones_row, recip1, start=True, stop=True)
            nc.vector.tensor_copy(recip_bc, ps_bc[:, 0:HR])

            # normalize into fp16: head 2ot in rows 0:64 (cols 0:8), head
            # 2ot+1 in rows 64:128 (cols 8:16)
            for ot in range(CT):
                nc.vector.tensor_tensor(
                    xcls16[0:HD, ot], xc_ps[0:HD, 16 * ot:16 * ot + R],
                    recip_bc[0:HD, 16 * ot:16 * ot + R], MULT)
                nc.vector.tensor_tensor(
                    xcls16[HD:P, ot], xc_ps[HD:P, 16 * ot + R:16 * ot + 16],
                    recip_bc[HD:P, 16 * ot + R:16 * ot + 16], MULT)

            # out-projection: outT[128, ot2, r] = sum_j WpT-slab.T @ xclsT
            for ot2 in range(CT):
                ps = ps_p.tile([P, 512], F32, tag="pp")
                po = ps[:, 0:R]
                for j in range(CT):
                    nc.tensor.matmul(
                        po,
                        wpt_sb[:, j, ot2 * P:(ot2 + 1) * P],
                        xcls16[:, j],
                        start=(j == 0), stop=(j == CT - 1))
                nc.vector.tensor_copy(outsb[:, ot2], po)
            nc.sync.dma_start(
                outt.rearrange("(j p) r -> p j r", p=P), outsb)

    nc.compile()
    return nc


def _prep_inputs(x, mask, Wq, Wk, Wv, Wp, bp):
    """Host-side sharding + layout prep. Returns per-core in_maps.

    The 8-token q projection and its fold through Wk (q2 = q*scale @
    Wk[head rows]) happen here: 76 MFLOP of the 312 GFLOP problem, and
    doing it on-device would force 9.4 MB of Wq/Wk DMA for 0.02% of the
    FLOPs."""
    x = np.asarray(x, dtype=np.float32)
    Wq = np.asarray(Wq, np.float32)
    Wk = np.asarray(Wk, np.float32)
    wvt = np.ascontiguousarray(np.asarray(Wv, np.float32).T.astype(np.float16))
    wpt = np.ascontiguousarray(np.asarray(Wp, np.float32).T.astype(np.float16))

    mask = np.asarray(mask)
    mask_full = np.empty((B, R, N), np.float32)
    mask_full[:, :, :R] = np.eye(R, dtype=np.float32)[None]
    mask_full[:, :, R:] = mask.astype(np.float32)

    # q2[b, hr, c] = sum_d q[b, r, h, d]*SCALE * Wk[h*HD+d, c]
    q = np.einsum('brc,dc->brd', x[:, :R], Wq) * SCALE        # [B, R, C]
    q2 = np.einsum('brhd,hdc->bhrc',
                   q.reshape(B, R, H, HD), Wk.reshape(H, HD, C))
    q2 = q2.reshape(B, HR, C)

    in_maps = []
    for b in range(B):
        xt_b = np.ascontiguousarray(x[b].T.astype(np.float16))
        xn_b = np.ascontiguousarray(x[b].astype(np.float16))
        q2t_b = np.zeros((C, HRP), np.float16)
        q2t_b[:, 0:HR] = q2[b].T.astype(np.float16)
        maskt_b = np.ascontiguousarray(mask_full[b].T)
        in_maps.append({
            "xt": xt_b, "xn": xn_b, "q2t": q2t_b, "maskt": maskt_b,
            "wvt": wvt, "wpt": wpt,
        })
    return in_maps


def _get_nc():
    if "nc" not in _RUNNER_CACHE:
        _RUNNER_CACHE["nc"] = _build()
    return _RUNNER_CACHE["nc"]


def kernel(x, mask, Wq, Wk, Wv, Wp, bp, repeats=8, **_unused):
    from concourse import bass_utils

    in_maps = _prep_inputs(x, mask, Wq, Wk, Wv, Wp, bp)
    nc = _get_nc()
    res = bass_utils.run_bass_kernel_spmd(nc, in_maps, core_ids=list(range(B)))
    out = np.stack(
        [res.results[b]["outt"].T for b in range(B)], axis=0)
    out = out + np.asarray(bp, np.float32).reshape(1, 1, C)
    return out.astype(np.float32)


if __name__ == "__main__":
    rng = np.random.default_rng(0)
    x = rng.standard_normal((B, N, C)).astype(np.float32)
    mask = rng.integers(0, 2, (B, R, N - R)) > 0
    s = 0.02
    Wq = (rng.standard_normal((C, C)) * s).astype(np.float32)
    Wk = (rng.standard_normal((C, C)) * s).astype(np.float32)
    Wv = (rng.standard_normal((C, C)) * s).astype(np.float32)
    Wp = (rng.standard_normal((C, C)) * s).astype(np.float32)
    bp = np.zeros(C, np.float32)
    out = kernel(x, mask, Wq, Wk, Wv, Wp, bp, 8)
    print("out", out.shape, out.dtype, np.abs(out).mean())


# revision 6
# speedup vs baseline: 1.1173x; 1.0554x over previous
"""AttentionPool kernel for 8x Trainium2 NeuronCores (Bass/Tile).

Problem (per batch b of B=8):
    q = (x[:, :8] @ Wq.T).reshape(8, 24, 64) * 64**-0.5
    k = (x @ Wk.T).reshape(4096, 24, 64)
    v = (x @ Wv.T).reshape(4096, 24, 64)
    attn = softmax(mask(q @ k.T))          # [24, 8, 4096]
    out = (attn @ v).reshape(8, 1536) @ Wp.T + bp

Sharding: data-parallel over B - one batch per NeuronCore, no collectives.

Key algebraic restructuring (R=8 queries makes pooling tiny):
  logits[h*8+r, n] = q2[h*8+r, :] . x[n, :]   with q2 = (q*scale) @ Wk[head
      rows] folded on the host (76 MFLOP) -> the 19.3 GFLOP K-projection
      becomes a 2.4 GFLOP GEMM against x directly.
  pool p[hr, :] = sum_n e[hr, n] x[n, :] (unnormalized, 2.4 GFLOP), then
      x_cls[r, hb] = p[h*8+r, :] @ WvT[:, hb] / den[hr]  (38 MFLOP)
      out = x_cls @ WpT (38 MFLOP) -> the 19.3 GFLOP V-projection vanishes.
  Total device FLOPs drop ~8x; the kernel becomes DMA-bound on streaming x
  in two layouts (c-major for logits stationary, token-major for pool
  stationary) in fp16, ~25 MB at the modeled 360 GB/s.

Schedule (DMA queue order == emission order; the stream is packed so the
DMA engines run gapless; WpT is loaded LAST so the pool/v-apply/normalize
tail hides under its transfer and out-proj co-streams with its arrival):
  per 512-token chunk: logits.T[tok, 192] per 128-token subtile (stationary
      = xT subtile, moving = q2T[ct]) -> exp (Act) -> * mask (DVE broadcast
      over heads) -> eT fp16; pool per c-tile: psum[c128, 192] accumulated
      over the chunk -> fp16 slabs pT (one per 2 chunks, copy+add drains);
      den via 1-col ones stationary after each chunk's pool.
  v-apply in 4 passes (after chunks 1/3/5/7): 12x12 matmuls of
      WvT-slab.T @ pT[:, ct, head-pair cols], all accumulating into ONE
      dedicated psum bank across passes (start only on the very first
      matmul; psum has_written bits make later regions/passes accumulate
      correctly) -> no SBUF accumulator traffic at all.
  tail: recip(den) broadcast via f32 matmul; normalize psum -> xclsT fp16
      (head 2t in rows 0:64/cols 0:8, head 2t+1 in rows 64:128/cols 8:16);
      out-proj per cout tile: psum[128, 8] = sum_j WpT-slab.T @ xclsT[:, j]
      -> outT[1536, 8] f32 -> host transposes + bias.
"""

import numpy as np

B, N, C = 8, 4096, 1536
H, HD, R = 24, 64, 8
HR = H * R           # 192 (h, r) pairs, index hr = h*R + r
HRP = 256            # q2t free-dim padded so DMA runs are 512B
SCALE = HD ** -0.5
P = 128
CT = C // P          # 12 contraction/output tiles
NCHUNK = 512
NSUB_CH = NCHUNK // P  # 4 subtiles per chunk
NT = N // NCHUNK     # 8 chunks
NSUB = N // P        # 32 token subtiles total

_RUNNER_CACHE = {}


def _build():
    import concourse.mybir as mybir
    import concourse.tile as tile
    from concourse import bacc

    F32 = mybir.dt.float32
    F16 = mybir.dt.float16
    MULT = mybir.AluOpType.mult
    EXP = mybir.ActivationFunctionType.Exp

    nc = bacc.Bacc(None, target_bir_lowering=False)
    xt = nc.dram_tensor("xt", [C, N], F16, kind="ExternalInput")      # x.T
    xn = nc.dram_tensor("xn", [N, C], F16, kind="ExternalInput")      # x
    q2t = nc.dram_tensor("q2t", [C, HRP], F16, kind="ExternalInput")  # q2.T
    maskt = nc.dram_tensor("maskt", [N, R], F32, kind="ExternalInput")
    wvt = nc.dram_tensor("wvt", [C, C], F16, kind="ExternalInput")    # Wv.T
    wpt = nc.dram_tensor("wpt", [C, C], F16, kind="ExternalInput")    # Wp.T
    outt = nc.dram_tensor("outt", [C, R], F32, kind="ExternalOutput")  # out.T

    with tile.TileContext(nc) as tc:
        with (
            tc.tile_pool(name="pper", bufs=1) as pper,      # persistent
            tc.tile_pool(name="pxt", bufs=3) as pxt,        # xT chunks
            tc.tile_pool(name="pxn", bufs=3) as pxn,        # x chunks
            tc.tile_pool(name="pwv", bufs=1) as pwv,
            tc.tile_pool(name="pwp", bufs=1) as pwp,
            tc.tile_pool(name="pexp", bufs=2) as pexp,
            tc.tile_pool(name="pslab", bufs=2) as pslab,
            tc.tile_pool(name="psmall", bufs=1) as psmall,
            tc.tile_pool(name="ps_l", bufs=2, space="PSUM") as ps_l,
            tc.tile_pool(name="ps_p", bufs=3, space="PSUM") as ps_p,
            tc.tile_pool(name="ps_d", bufs=1, space="PSUM") as ps_d,
            tc.tile_pool(name="ps_x", bufs=1, space="PSUM") as ps_x,
        ):
            # ---------- persistent tiles ----------
            q2t_sb = pper.tile([P, CT, HRP], F16, tag="q2t")
            maskt_sb = pper.tile([P, NSUB, R], F32, tag="maskt")
            eT = pper.tile([P, NSUB, HR], F16, tag="eT")        # masked exp
            den_acc = pper.tile([1, HR], F32, tag="den")
            ones16 = pper.tile([P, 1], F16, tag="ones16")
            ones_row = pper.tile([1, P], F32, tag="onesrow")
            recip_bc = pper.tile([P, HR], F32, tag="recip")
            xcls16 = pper.tile([P, CT, R], F16, tag="xcls")
            outsb = pper.tile([P, CT, R], F32, tag="outsb")
            # single psum bank accumulating x_cls across all 4 v-apply passes
            xc_ps = ps_x.tile([P, 512], F32, tag="px")

            # ---------- DMA emission helpers (order == queue order) -------
            xt_ch0 = pxt.tile([P, CT, NCHUNK], F16, tag="xt")

            def _xt_half(xt_ch, nt, half):
                lo = nt * NCHUNK + half * (NCHUNK // 2)
                nc.sync.dma_start(
                    xt_ch[:, :, half * (NCHUNK // 2):(half + 1) * (NCHUNK // 2)],
                    xt[:, lo:lo + NCHUNK // 2].rearrange(
                        "(ct p) n -> p ct n", p=P))

            xn_ch0 = pxn.tile([P, NSUB_CH, C], F16, tag="xn")

            def _xn_half(xn_ch, nt, half):
                lo = nt * NCHUNK + half * (NCHUNK // 2)
                nc.sync.dma_start(
                    xn_ch[:, half * 2:(half + 1) * 2],
                    xn[lo:lo + NCHUNK // 2, :].rearrange(
                        "(s p) c -> p s c", p=P))

            wvt_sb = pwv.tile([P, CT, C], F16, tag="wv")
            wpt_sb = pwp.tile([P, CT, C], F16, tag="wp")

            def _w_slab(dst_sb, src, quarter):
                w4 = C // 4
                nc.sync.dma_start(
                    dst_sb[:, :, quarter * w4:(quarter + 1) * w4],
                    src[:, quarter * w4:(quarter + 1) * w4].rearrange(
                        "(ct p) o -> p ct o", p=P))

            # startup: chunk 0 + q2t + mask, then Wv (needed by the first
            # v-apply pass after chunk 1). WpT is NOT here - it loads at the
            # very end of the stream.
            _xt_half(xt_ch0, 0, 0)
            nc.sync.dma_start(
                q2t_sb, q2t.rearrange("(ct p) hr -> p ct hr", p=P))
            _xt_half(xt_ch0, 0, 1)
            nc.sync.dma_start(
                maskt_sb, maskt.rearrange("(s p) r -> p s r", p=P))
            _xn_half(xn_ch0, 0, 0)
            _xn_half(xn_ch0, 0, 1)
            for quarter in range(4):
                _w_slab(wvt_sb, wvt, quarter)

            # ones vectors (fp16 via copy from f32 memset)
            ones_f = psmall.tile([P, 1], F32, tag="onesf")
            nc.vector.memset(ones_f, 1.0)
            nc.vector.tensor_copy(ones16, ones_f)
            nc.vector.memset(ones_row, 1.0)

            # ---------- per-chunk pipeline ----------
            def emit_logits(nt, xt_ch):
                for s in range(NSUB_CH):
                    si = nt * NSUB_CH + s
                    ps = ps_l.tile([P, 512], F32, tag="pl")
                    lT = ps[:, 0:HR]
                    for ct in range(CT):
                        nc.tensor.matmul(
                            lT,
                            xt_ch[:, ct, s * P:(s + 1) * P],
                            q2t_sb[:, ct, 0:HR],
                            start=(ct == 0), stop=(ct == CT - 1))
                    exp_f = pexp.tile([P, HR], F32, tag="expf")
                    nc.scalar.activation(exp_f, lT, EXP)
                    nc.vector.tensor_tensor(
                        eT[:, si].rearrange("p (h r) -> p h r", h=H),
                        exp_f.rearrange("p (h r) -> p h r", h=H),
                        maskt_sb[:, si, None, :].to_broadcast((P, H, R)),
                        MULT)

            def emit_pool(nt, xn_ch):
                slab = pslab.tile([P, CT, HR], F16, tag="slab")
                for ct in range(CT):
                    ps = ps_p.tile([P, 512], F32, tag="pp")
                    pch = ps[:, 0:HR]
                    for s in range(NSUB_CH):
                        si = nt * NSUB_CH + s
                        nc.tensor.matmul(
                            pch,
                            xn_ch[:, s, ct * P:(ct + 1) * P],
                            eT[:, si],
                            start=(s == 0), stop=(s == NSUB_CH - 1))
                    # drains split across DVE and Act so neither throttles
                    # the pool's psum rotation
                    if ct % 2 == 0:
                        nc.vector.tensor_copy(slab[:, ct], pch)
                    else:
                        nc.scalar.copy(slab[:, ct], pch)
                return slab

            def emit_den(nt):
                den_ch = ps_d.tile([P, 512], F32, tag="pd")
                for s in range(NSUB_CH):
                    si = nt * NSUB_CH + s
                    nc.tensor.matmul(
                        den_ch[0:1, 0:HR], ones16, eT[:, si],
                        start=(s == 0), stop=(s == NSUB_CH - 1))
                if nt == 0:
                    nc.vector.tensor_copy(den_acc, den_ch[0:1, 0:HR])
                else:
                    nc.vector.tensor_add(
                        den_acc, den_acc, den_ch[0:1, 0:HR])

            def emit_vapply(nt, slab):
                # x_cls accumulation into one dedicated psum bank, all 8
                # chunks. Half-column groups: head 2ot -> psum rows 0:64,
                # head 2ot+1 -> rows 64:128, both in columns 8ot..8ot+8, so
                # the final normalize is 2 contiguous DVE ops. start only on
                # the very first matmul (clears the bank once); every other
                # region/pass relies on psum has_written bits: first write
                # to a fresh region replaces, later ones accumulate.
                for ot in range(CT):
                    for half in range(2):
                        pv = xc_ps[half * HD:(half + 1) * HD,
                                   8 * ot:8 * ot + 8]
                        for ct in range(CT):
                            nc.tensor.matmul(
                                pv,
                                wvt_sb[:, ct,
                                       ot * P + half * HD:
                                       ot * P + (half + 1) * HD],
                                slab[:, ct,
                                     16 * ot + half * R:
                                     16 * ot + (half + 1) * R],
                                start=(nt == 0 and ot == 0 and half == 0
                                       and ct == 0),
                                stop=(nt == NT - 1 and ct == CT - 1),
                                skip_group_check=True)

            for nt in range(NT):
                if nt == 0:
                    xt_ch, xn_ch = xt_ch0, xn_ch0
                else:
                    xt_ch = pxt.tile([P, CT, NCHUNK], F16, tag="xt")
                    _xt_half(xt_ch, nt, 0)
                    _xt_half(xt_ch, nt, 1)
                    xn_ch = pxn.tile([P, NSUB_CH, C], F16, tag="xn")
                    _xn_half(xn_ch, nt, 0)
                    _xn_half(xn_ch, nt, 1)
                emit_logits(nt, xt_ch)
                slab = emit_pool(nt, xn_ch)
                emit_den(nt)
                emit_vapply(nt, slab)

            # WpT loads at the very end of the DMA stream: the whole
            # pool/v-apply/normalize tail hides under its transfer and the
            # out-projection co-streams with its column quarters.
            for quarter in range(4):
                _w_slab(wpt_sb, wpt, quarter)

            # ---------- tail ----------
            recip1 = psmall.tile([1, HR], F32, tag="recip1")
            nc.vector.reciprocal(recip1, den_acc)
            ps_bc = ps_d.tile([P, 512], F32, tag="pd")
            nc.tensor.matmul(
                ps_bc[:, 0:HR], ones_row, recip1, start=True, stop=True)
            nc.vector.tensor_copy(recip_bc, ps_bc[:, 0:HR])

            # normalize into fp16: 2 contiguous ops; the recip row-half
            # views pick head 2ot (cols 16ot+r) resp. 2ot+1 (16ot+8+r)
            rbc = recip_bc.rearrange("p (t s) -> p t s", t=CT)
            xcv = xc_ps[:, 0:CT * R].rearrange("p (t r) -> p t r", t=CT)
            nc.vector.tensor_tensor(
                xcls16[0:HD], xcv[0:HD], rbc[0:HD, :, 0:R], MULT)
            nc.vector.tensor_tensor(
                xcls16[HD:P], xcv[HD:P], rbc[HD:P, :, R:2 * R], MULT)

            # out-projection accumulated in the (reused) dedicated psum
            # bank: outT[128, 8*ot2+r] = sum_j WpT-slab.T @ xclsT[:, j]
            wp_ps = ps_x.tile([P, 512], F32, tag="px")
            for ot2 in range(CT):
                po = wp_ps[:, 8 * ot2:8 * ot2 + 8]
                for j in range(CT):
                    nc.tensor.matmul(
                        po,
                        wpt_sb[:, j, ot2 * P:(ot2 + 1) * P],
                        xcls16[:, j],
                        start=(ot2 == 0 and j == 0),
                        stop=(ot2 == CT - 1 and j == CT - 1),
                        skip_group_check=True)
            nc.vector.tensor_copy(
                outsb.rearrange("p t r -> p (t r)"), wp_ps[:, 0:CT * R])
            nc.sync.dma_start(
                outt.rearrange("(j p) r -> p j r", p=P), outsb)

    nc.compile()
    return nc


def _prep_inputs(x, mask, Wq, Wk, Wv, Wp, bp):
    """Host-side sharding + layout prep. Returns per-core in_maps.

    The 8-token q projection and its fold through Wk (q2 = q*scale @
    Wk[head rows]) happen here: 76 MFLOP of the 312 GFLOP problem, and
    doing it on-device would force 9.4 MB of Wq/Wk DMA for 0.02% of the
    FLOPs."""
    x = np.asarray(x, dtype=np.float32)
    Wq = np.asarray(Wq, np.float32)
    Wk = np.asarray(Wk, np.float32)
    wvt = np.ascontiguousarray(np.asarray(Wv, np.float32).T.astype(np.float16))
    wpt = np.ascontiguousarray(np.asarray(Wp, np.float32).T.astype(np.float16))

    mask = np.asarray(mask)
    mask_full = np.empty((B, R, N), np.float32)
    mask_full[:, :, :R] = np.eye(R, dtype=np.float32)[None]
    mask_full[:, :, R:] = mask.astype(np.float32)

    # q2[b, hr, c] = sum_d q[b, r, h, d]*SCALE * Wk[h*HD+d, c]
    q = np.einsum('brc,dc->brd', x[:, :R], Wq) * SCALE        # [B, R, C]
    q2 = np.einsum('brhd,hdc->bhrc',
                   q.reshape(B, R, H, HD), Wk.reshape(H, HD, C))
    q2 = q2.reshape(B, HR, C)

    in_maps = []
    for b in range(B):
        xt_b = np.ascontiguousarray(x[b].T.astype(np.float16))
        xn_b = np.ascontiguousarray(x[b].astype(np.float16))
        q2t_b = np.zeros((C, HRP), np.float16)
        q2t_b[:, 0:HR] = q2[b].T.astype(np.float16)
        maskt_b = np.ascontiguousarray(mask_full[b].T)
        in_maps.append({
            "xt": xt_b, "xn": xn_b, "q2t": q2t_b, "maskt": maskt_b,
            "wvt": wvt, "wpt": wpt,
        })
    return in_maps


def _get_nc():
    if "nc" not in _RUNNER_CACHE:
        _RUNNER_CACHE["nc"] = _build()
    return _RUNNER_CACHE["nc"]


def kernel(x, mask, Wq, Wk, Wv, Wp, bp, repeats=8, **_unused):
    from concourse import bass_utils

    in_maps = _prep_inputs(x, mask, Wq, Wk, Wv, Wp, bp)
    nc = _get_nc()
    res = bass_utils.run_bass_kernel_spmd(nc, in_maps, core_ids=list(range(B)))
    out = np.stack(
        [res.results[b]["outt"].T for b in range(B)], axis=0)
    out = out + np.asarray(bp, np.float32).reshape(1, 1, C)
    return out.astype(np.float32)


if __name__ == "__main__":
    rng = np.random.default_rng(0)
    x = rng.standard_normal((B, N, C)).astype(np.float32)
    mask = rng.integers(0, 2, (B, R, N - R)) > 0
    s = 0.02
    Wq = (rng.standard_normal((C, C)) * s).astype(np.float32)
    Wk = (rng.standard_normal((C, C)) * s).astype(np.float32)
    Wv = (rng.standard_normal((C, C)) * s).astype(np.float32)
    Wp = (rng.standard_normal((C, C)) * s).astype(np.float32)
    bp = np.zeros(C, np.float32)
    out = kernel(x, mask, Wq, Wk, Wv, Wp, bp, 8)
    print("out", out.shape, out.dtype, np.abs(out).mean())
